# revision 1
# baseline (speedup 1.0000x reference)
"""CCX loss kernel for Trainium2 (8 NeuronCores, data-parallel over batch).

Math (per batch element n, with C=256 channels, HW=64*64=4096 pixels):
  y_mu[c]   = mean over (n, h, w) of y            (host, tiny)
  x_c = x - y_mu ; y_c = y - y_mu                 (device)
  x_n = x_c/||x_c||_C ; y_n = y_c/||y_c||_C       (device)
  s[i,j]    = sum_c x_n[c,i] y_n[c,j]             (device matmul, f32r)
  d = 1-s ; dt = d/(dmin_i+eps) ; w = exp((1-dt)/0.5)
  ccx_ij = w/sum_j w ; ccx_n = mean_j max_i ccx_ij
  loss = mean_n -log(ccx_n + eps)                 (host, 8 scalars)

Key identities used on device:
  w_ij = exp(s*a_i + b_i),  a_i = 2/(dmin_i+eps), b_i = 2-a_i
  s*a_i = G*alpha_i with G = x_c^T y_n (unnormalized-x matmul),
          alpha_i = a_i/||x_c[:,i]||
  max_i ccx_ij = exp(max_i (G^T[j,i]*alpha_i + (b_i - lnZ_i)))
  -> pass 2 computes A = (x_c*alpha)^T-style matmul + K=1 bias-row matmul,
     then a plain free-dim max per j-block.
"""

import os
import sys

import numpy as np

sys.path.insert(0, "/opt/trn_rl_repo")
os.environ.setdefault("JAX_PLATFORMS", "axon")

import concourse.mybir as mybir
import concourse.tile as tile
from concourse import bacc, bass_isa
from concourse.bass_utils import run_bass_kernel_spmd
from concourse.masks import make_identity

N, C, H, W = 8, 256, 64, 64
HW = H * W          # 4096
NB = HW // 128      # 32 blocks of 128 rows/cols
NQ = 4              # psum quarters per block
QW = HW // NB // 1  # 128
QUARTER = 1024      # quarter width (2 psum banks)
EPS = 1e-6
F32 = mybir.dt.float32
F32R = mybir.dt.float32r
ALU = mybir.AluOpType
ACTF = mybir.ActivationFunctionType

_cached = {}


def _build():
    nc = bacc.Bacc(None, target_bir_lowering=False, debug=True)
    xs = nc.dram_tensor("xs", [C, HW], F32, kind="ExternalInput")
    ys = nc.dram_tensor("ys", [C, HW], F32, kind="ExternalInput")
    ymu = nc.dram_tensor("ymu", [128, 2], F32, kind="ExternalInput")
    out = nc.dram_tensor("out", [1, 1], F32, kind="ExternalOutput")
    scr_y = nc.dram_tensor("scr_y", [NB, 128], F32)
    scr_a = nc.dram_tensor("scr_a", [NB, 128], F32)
    scr_b = nc.dram_tensor("scr_b", [NB, 128], F32)

    repeat = int(os.environ.get("BASS_REPEAT", "1"))
    phase = int(os.environ.get("BASS_PHASE", "3"))
    with tile.TileContext(nc) as tc:
        import contextlib
        rep_ctx = tc.For_i(0, repeat, 1) if repeat > 1 else contextlib.nullcontext()
        with rep_ctx:
         with (
             tc.tile_pool(name="big", bufs=1) as big,
             tc.tile_pool(name="bc3", bufs=3) as bc3,
             tc.tile_pool(name="sq", bufs=2) as sqp,
             tc.tile_pool(name="small", bufs=1) as sm,
             tc.tile_pool(name="mmq", bufs=8, space="PSUM") as mmq,
         ):
             # ---------------- load ----------------
             x = big.tile([128, 2, HW], F32, tag="x")
             xc = big.tile([128, 2, HW], F32R, tag="xc")
             y = big.tile([128, 2, HW], F32, tag="y")
             yn = big.tile([128, 2, HW], F32R, tag="yn")
             ymu_sb = sm.tile([128, 2], F32, tag="ymu")
             nc.sync.dma_start(out=x[:, :, :], in_=xs.rearrange("(g p) j -> p g j", p=128))
             nc.sync.dma_start(out=y[:, :, :], in_=ys.rearrange("(g p) j -> p g j", p=128))
             nc.sync.dma_start(out=ymu_sb[:, :], in_=ymu[:, :])

             ones_col = sm.tile([128, 1], F32, tag="ones_col")
             nc.vector.memset(ones_col[:, :], 1.0)
             ones_row_f = sm.tile([1, 128], F32, tag="ones_row_f")
             nc.vector.memset(ones_row_f[:, :], 1.0)
             ones_row_r = sm.tile([1, 128], F32R, tag="ones_row_r")
             nc.vector.tensor_scalar(
                 out=ones_row_r[:, :], in0=ones_row_f[:, :], scalar1=1.0,
                 scalar2=None, op0=ALU.mult)

             # ---------------- center (in place; x rounds to f32r) --------
             for g in range(2):
                 nc.vector.tensor_scalar(
                     out=xc[:, g, :], in0=x[:, g, :],
                     scalar1=ymu_sb[:, g : g + 1], scalar2=None, op0=ALU.subtract)
                 nc.vector.tensor_scalar(
                     out=y[:, g, :], in0=y[:, g, :],
                     scalar1=ymu_sb[:, g : g + 1], scalar2=None, op0=ALU.subtract)

             # ---------------- channel sumsq -> 1/norm (col layout) -------
             # pscol[:, r] (x) / pscol[:, 32+r] (y): per-pixel sum over C of sq
             pscol = mmq.tile([128, 64], F32, tag="pq")
             for ti, src in ((0, xc), (1, y)):
                 for ch in range(4):
                     sqs = []
                     for g in range(2):
                         sq = sqp.tile([128, 1024], F32, tag="sqt")
                         nc.scalar.activation(
                             out=sq[:, :],
                             in_=src[:, g, 1024 * ch : 1024 * (ch + 1)].bitcast(F32),
                             func=ACTF.Square)
                         sqs.append(sq)
                     for k in range(8):
                         r = 8 * ch + k
                         for g in range(2):
                             nc.tensor.matmul(
                                 pscol[:, 32 * ti + r : 32 * ti + r + 1],
                                 sqs[g][:, 128 * k : 128 * (k + 1)],
                                 ones_col[:, :],
                                 start=(g == 0), stop=(g == 1))
             norms = sm.tile([128, 64], F32, tag="norms")
             nc.scalar.activation(out=norms[:, :], in_=pscol[:, :], func=ACTF.Sqrt)
             invc = sm.tile([128, 64], F32, tag="invc")
             nc.vector.reciprocal(invc[:, :], norms[:, :])
             # invx = invc[:, 0:32], invy = invc[:, 32:64]

             # ---------------- broadcast invy along partitions -------------
             # col->DRAM (transposing dst AP), then DRAM->SBUF partition-bcast
             import concourse.bass as bass_mod
             nc.sync.dma_start(
                 out=scr_y[:, :].rearrange("r p -> p r"), in_=invc[:, 32:64])
             invybc = bc3.tile([128, HW], F32, tag="bcast")
             bcast_src_y = bass_mod.AP(
                 tensor=scr_y[:, :].tensor, offset=0, ap=[[0, 128], [1, HW]])
             nc.sync.dma_start(out=invybc[:, :], in_=bcast_src_y)

             # ---------------- y_n = y_c * invy (f32r) ---------------------
             for g in range(2):
                 nc.vector.tensor_tensor(
                     out=yn[:, g, :], in0=y[:, g, :], in1=invybc[:, :],
                     op=ALU.mult)

             # ---------------- PASS 1: row max + Z -------------------------
             gacc = sm.tile([128, 256], F32, tag="gacc")
             zacc = sm.tile([128, 256], F32, tag="zacc")
             gmaxc = sm.tile([128, 32], F32, tag="gmaxc")
             reccol = sm.tile([128, 32], F32, tag="reccol")
             ab64 = sm.tile([128, 64], F32, tag="ab64")  # alpha | b2
             bcol = sm.tile([128, 32], F32, tag="bcol")
             tmpc = sm.tile([128, 32], F32, tag="tmpc")

             for r in range(NB if phase >= 1 else 0):
                 pqs = [mmq.tile([128, 512], F32, tag="pq", name=f"pq_{_i}") for _i in range(8)]
                 for cp in range(4):
                     c0, c1 = 2 * cp, 2 * cp + 1
                     for g in range(2):
                         for c in (c0, c1):
                             nc.tensor.matmul(
                                 pqs[c][:, :],
                                 xc[:, g, 128 * r : 128 * (r + 1)],
                                 yn[:, g, 512 * c : 512 * (c + 1)],
                                 start=(g == 0), stop=(g == 1))
                     for c in (c0, c1):
                         nc.vector.reduce_max(
                             gacc[:, 8 * r + c : 8 * r + c + 1], pqs[c][:, :],
                             axis=mybir.AxisListType.X)
                 # alpha chain for this block
                 nc.vector.reduce_max(
                     gmaxc[:, r : r + 1],
                     gacc[:, 8 * r : 8 * r + 8], axis=mybir.AxisListType.X)
                 # smax = gmax * invx_r ; dminE = 1+eps - smax
                 nc.vector.tensor_scalar(
                     out=tmpc[:, r : r + 1], in0=gmaxc[:, r : r + 1],
                     scalar1=invc[:, r : r + 1], scalar2=None, op0=ALU.mult)
                 nc.vector.tensor_scalar(
                     out=tmpc[:, r : r + 1], in0=tmpc[:, r : r + 1],
                     scalar1=-1.0, scalar2=float(1.0 + EPS),
                     op0=ALU.mult, op1=ALU.add)
                 nc.vector.reciprocal(reccol[:, r : r + 1], tmpc[:, r : r + 1])
                 nc.vector.tensor_scalar(
                     out=ab64[:, r : r + 1], in0=reccol[:, r : r + 1],
                     scalar1=invc[:, r : r + 1], scalar2=2.0,
                     op0=ALU.mult, op1=ALU.mult)
                 nc.vector.tensor_scalar(
                     out=bcol[:, r : r + 1], in0=reccol[:, r : r + 1],
                     scalar1=-2.0, scalar2=2.0, op0=ALU.mult, op1=ALU.add)
                 for c in range(8):
                     nc.scalar.activation(
                         out=pqs[c][:, :], in_=pqs[c][:, :], func=ACTF.Exp,
                         bias=bcol[:, r : r + 1],
                         scale=ab64[:, r : r + 1],
                         accum_out=zacc[:, 8 * r + c : 8 * r + c + 1])

             # ---------------- interlude: b2 = b1 - lnZ; broadcasts --------
             if phase >= 2:
              zsum = sm.tile([128, 32], F32, tag="zsum")
              nc.vector.reduce_sum(
                  zsum[:, :], zacc[:, :].rearrange("p (r q) -> p r q", q=8),
                  axis=mybir.AxisListType.X)
              lnz = sm.tile([128, 32], F32, tag="lnz")
              nc.scalar.activation(out=lnz[:, :], in_=zsum[:, :], func=ACTF.Ln)
              nc.vector.tensor_tensor(
                  out=ab64[:, 32:64], in0=bcol[:, :], in1=lnz[:, :], op=ALU.subtract)

              # alpha/b2 cols -> DRAM rows (transposing dst), then bcast/row
              nc.sync.dma_start(
                  out=scr_a[:, :].rearrange("r p -> p r"), in_=ab64[:, 0:32])
              nc.sync.dma_start(
                  out=scr_b[:, :].rearrange("r p -> p r"), in_=ab64[:, 32:64])
              abc = bc3.tile([128, HW], F32, tag="bcast")
              bcast_src_a = bass_mod.AP(
                  tensor=scr_a[:, :].tensor, offset=0, ap=[[0, 128], [1, HW]])
              nc.sync.dma_start(out=abc[:, :], in_=bcast_src_a)
              b2row_f = bc3.tile([1, HW], F32, tag="bcast")
              nc.sync.dma_start(
                  out=b2row_f[0:1, :],
                  in_=scr_b[:, :].rearrange("r p -> (r p)"))
              b2row = bc3.tile([1, HW], F32R, tag="bcast")
              nc.vector.tensor_scalar(
                  out=b2row[:, :], in0=b2row_f[:, :], scalar1=1.0, scalar2=None,
                  op0=ALU.mult)

              # x2 = x_c * alpha  (in place, f32r)
              for g in range(2):
                  nc.vector.tensor_tensor(
                      out=xc[:, g, :], in0=xc[:, g, :].bitcast(F32),
                      in1=abc[:, :], op=ALU.mult)

             # ---------------- PASS 2: col max of A ------------------------
             macc = sm.tile([128, 256], F32, tag="macc")
             if phase < 3:
                 nc.vector.memset(macc[:, :], -1.0)
             for rb in range(NB if phase >= 3 else 0):
                 pqs = [mmq.tile([128, 512], F32, tag="pq", name=f"pq_{_i}") for _i in range(8)]
                 for c in range(8):
                     nc.tensor.matmul(
                         pqs[c][:, :], ones_row_r[:, :],
                         b2row[:, 512 * c : 512 * (c + 1)],
                         start=True, stop=False)
                 for c in range(8):
                     nc.tensor.matmul(
                         pqs[c][:, :],
                         yn[:, 0, 128 * rb : 128 * (rb + 1)],
                         xc[:, 0, 512 * c : 512 * (c + 1)],
                         start=False, stop=False)
                 for c in range(8):
                     nc.tensor.matmul(
                         pqs[c][:, :],
                         yn[:, 1, 128 * rb : 128 * (rb + 1)],
                         xc[:, 1, 512 * c : 512 * (c + 1)],
                         start=False, stop=True)
                     nc.vector.reduce_max(
                         macc[:, 8 * rb + c : 8 * rb + c + 1], pqs[c][:, :],
                         axis=mybir.AxisListType.X)

             # ---------------- final ---------------------------------------
             mcol = sm.tile([128, 32], F32, tag="mcol")
             nc.vector.reduce_max(
                 mcol[:, :], macc[:, :].rearrange("p (r q) -> p r q", q=8),
                 axis=mybir.AxisListType.X)
             expm = sm.tile([128, 32], F32, tag="expm")
             csum = sm.tile([128, 1], F32, tag="csum")
             nc.scalar.activation(
                 out=expm[:, :], in_=mcol[:, :], func=ACTF.Exp,
                 accum_out=csum[:, :])
             tot = sm.tile([128, 1], F32, tag="tot")
             nc.gpsimd.partition_all_reduce(
                 tot[:, :], csum[:, :], channels=128,
                 reduce_op=bass_isa.ReduceOp.add)
             res = sm.tile([1, 1], F32, tag="res")
             nc.vector.tensor_scalar(
                 out=res[:, :], in0=tot[0:1, :], scalar1=float(1.0 / HW),
                 scalar2=None, op0=ALU.mult)
             nc.sync.dma_start(out=out[:, :], in_=res[:, :])
    nc.compile()
    return nc


def _get_nc():
    if "nc" not in _cached:
        _cached["nc"] = _build()
    return _cached["nc"]


def run_device(x, y, trace=False):
    """x, y: (N, C, H, W) float32. Returns (ccx (N,), BassKernelResults)."""
    x = np.ascontiguousarray(np.asarray(x, dtype=np.float32))
    y = np.ascontiguousarray(np.asarray(y, dtype=np.float32))
    ymu = y.mean(axis=(0, 2, 3), dtype=np.float64).astype(np.float32)  # (C,)
    ymu_arr = np.ascontiguousarray(ymu.reshape(2, 128).T)  # (128, 2)
    in_maps = []
    for n in range(N):
        in_maps.append({
            "xs": np.ascontiguousarray(x[n].reshape(C, HW)),
            "ys": np.ascontiguousarray(y[n].reshape(C, HW)),
            "ymu": ymu_arr,
        })
    nc = _get_nc()
    res = run_bass_kernel_spmd(nc, in_maps, core_ids=list(range(N)), trace=trace)
    ccx = np.array([res.results[n]["out"][0, 0] for n in range(N)], dtype=np.float32)
    return ccx, res


def kernel(x, y):
    ccx, _ = run_device(x, y)
    loss = float(np.mean(-np.log(ccx.astype(np.float64) + EPS)))
    return np.float32(loss)


if __name__ == "__main__":
    rng = np.random.default_rng(0)
    x = rng.standard_normal((N, C, H, W), dtype=np.float32)
    y = rng.standard_normal((N, C, H, W), dtype=np.float32)
    print("loss:", kernel(x, y))



# revision 3
# speedup vs baseline: 1.1561x; 1.1561x over previous
"""CCX loss kernel for Trainium2 (8 NeuronCores, data-parallel over batch).

Math (per batch element n, with C=256 channels, HW=64*64=4096 pixels):
  y_mu[c]   = mean over (n, h, w) of y            (host, tiny)
  x_c = x - y_mu ; y_c = y - y_mu                 (device)
  x_n = x_c/||x_c||_C ; y_n = y_c/||y_c||_C       (device)
  s[i,j]    = sum_c x_n[c,i] y_n[c,j]             (device matmul, f32r)
  d = 1-s ; dt = d/(dmin_i+eps) ; w = exp((1-dt)/0.5)
  ccx_ij = w/sum_j w ; ccx_n = mean_j max_i ccx_ij
  loss = mean_n -log(ccx_n + eps)                 (host, 8 scalars)

Key identities used on device:
  w_ij = exp(s*a_i + b_i),  a_i = 2/(dmin_i+eps), b_i = 2-a_i
  s*a_i = G*alpha_i with G = x_c^T y_n (unnormalized-x matmul),
          alpha_i = a_i/||x_c[:,i]||
  max_i ccx_ij = exp(max_i (G^T[j,i]*alpha_i + (b_i - lnZ_i)))
  -> pass 2 computes A = (x_c*alpha)^T-style matmul + K=1 bias-row matmul,
     then a plain free-dim max per j-block.

Schedule notes (perf):
  - PSUM is managed as 4 tiles of [128, 1024] (2 banks each); pass-1 row
    stats (reduce_max), the exp, and pass-2 maxes all run at FD=1024 to
    amortize per-instruction overhead while keeping the PE's bank-reuse
    stall under the ~3.4us HAM re-throttle window.
  - Matmuls are ordered stationary-major (all 8 chunks per weight load)
    so LDWEIGHTS can be pulled ahead / overlapped.
  - Centering and squaring run on the scalar engine (bias=-y_mu fused),
    keeping the vector engine free for the reduce_max sweeps.
"""

import os
import sys

import numpy as np

sys.path.insert(0, "/opt/trn_rl_repo")
os.environ.setdefault("JAX_PLATFORMS", "axon")

import concourse.mybir as mybir
import concourse.tile as tile
from concourse import bacc, bass_isa
from concourse.bass_utils import run_bass_kernel_spmd

N, C, H, W = 8, 256, 64, 64
HW = H * W          # 4096
NB = HW // 128      # 32 blocks of 128 rows/cols
NT = 4              # psum tiles per block (each [128, 1024] = 2 banks)
TW = HW // NB // 1  # 128
EPS = 1e-6
F32 = mybir.dt.float32
F32R = mybir.dt.float32r
ALU = mybir.AluOpType
ACTF = mybir.ActivationFunctionType
AX = mybir.AxisListType

_cached = {}


def _build():
    nc = bacc.Bacc(None, target_bir_lowering=False, debug=True)
    xs = nc.dram_tensor("xs", [C, HW], F32, kind="ExternalInput")
    ys = nc.dram_tensor("ys", [C, HW], F32, kind="ExternalInput")
    ymu = nc.dram_tensor("ymu", [128, 2], F32, kind="ExternalInput")
    out = nc.dram_tensor("out", [1, 1], F32, kind="ExternalOutput")
    scr_y = nc.dram_tensor("scr_y", [NB, 128], F32)
    scr_a = nc.dram_tensor("scr_a", [NB, 128], F32)
    scr_b = nc.dram_tensor("scr_b", [NB, 128], F32)

    repeat = int(os.environ.get("BASS_REPEAT", "1"))
    with tile.TileContext(nc) as tc:
        import contextlib
        rep_ctx = tc.For_i(0, repeat, 1) if repeat > 1 else contextlib.nullcontext()
        with rep_ctx:
         with (
             tc.tile_pool(name="big", bufs=1) as big,
             tc.tile_pool(name="bc3", bufs=3) as bc3,
             tc.tile_pool(name="sq", bufs=2) as sqp,
             tc.tile_pool(name="small", bufs=1) as sm,
             tc.tile_pool(name="mmq", bufs=4, space="PSUM") as mmq,
         ):
             # ---------------- load ----------------
             x = big.tile([128, 2, HW], F32, tag="x")
             xc = big.tile([128, 2, HW], F32R, tag="xc")
             y = big.tile([128, 2, HW], F32, tag="y")
             yn = big.tile([128, 2, HW], F32R, tag="yn")
             ymu_sb = sm.tile([128, 2], F32, tag="ymu")
             nc.sync.dma_start(out=x[:, :, :], in_=xs.rearrange("(g p) j -> p g j", p=128))
             nc.sync.dma_start(out=y[:, :, :], in_=ys.rearrange("(g p) j -> p g j", p=128))
             nc.sync.dma_start(out=ymu_sb[:, :], in_=ymu[:, :])

             ones_col = sm.tile([128, 1], F32, tag="ones_col")
             nc.vector.memset(ones_col[:, :], 1.0)
             ones_row_f = sm.tile([1, 128], F32, tag="ones_row_f")
             nc.vector.memset(ones_row_f[:, :], 1.0)
             ones_row_r = sm.tile([1, 128], F32R, tag="ones_row_r")
             nc.vector.tensor_scalar(
                 out=ones_row_r[:, :], in0=ones_row_f[:, :], scalar1=1.0,
                 scalar2=None, op0=ALU.mult)
             negymu = sm.tile([128, 2], F32, tag="negymu")
             nc.vector.tensor_scalar(
                 out=negymu[:, :], in0=ymu_sb[:, :], scalar1=-1.0,
                 scalar2=None, op0=ALU.mult)

             # ---------------- center on ACT (bias = -ymu) -----------------
             for g in range(2):
                 nc.scalar.activation(
                     out=xc[:, g, :], in_=x[:, g, :],
                     func=ACTF.Identity, bias=negymu[:, g : g + 1])
                 nc.scalar.activation(
                     out=y[:, g, :], in_=y[:, g, :],
                     func=ACTF.Identity, bias=negymu[:, g : g + 1])

             # ---------------- channel sumsq -> 1/norm (col layout) -------
             # pscol[:, r] (x) / pscol[:, 32+r] (y): per-pixel sum over C of sq
             # sq = Square(raw + (-ymu)) reads the raw tensors directly.
             pscol = mmq.tile([128, 64], F32, tag="pq")
             for ti, src in ((0, x), (1, y)):
                 for ch in range(4):
                     sqs = []
                     for g in range(2):
                         sq = sqp.tile([128, 1024], F32, tag="sqt")
                         if ti == 0:
                             nc.scalar.activation(
                                 out=sq[:, :],
                                 in_=src[:, g, 1024 * ch : 1024 * (ch + 1)],
                                 func=ACTF.Square,
                                 bias=negymu[:, g : g + 1])
                         else:
                             # y is already centered in place
                             nc.scalar.activation(
                                 out=sq[:, :],
                                 in_=src[:, g, 1024 * ch : 1024 * (ch + 1)],
                                 func=ACTF.Square)
                         sqs.append(sq)
                     for k in range(8):
                         r = 8 * ch + k
                         for g in range(2):
                             nc.tensor.matmul(
                                 pscol[:, 32 * ti + r : 32 * ti + r + 1],
                                 sqs[g][:, 128 * k : 128 * (k + 1)],
                                 ones_col[:, :],
                                 start=(g == 0), stop=(g == 1))
             norms = sm.tile([128, 64], F32, tag="norms")
             nc.scalar.activation(out=norms[:, :], in_=pscol[:, :], func=ACTF.Sqrt)
             invc = sm.tile([128, 64], F32, tag="invc")
             nc.vector.reciprocal(invc[:, :], norms[:, :])
             # invx = invc[:, 0:32], invy = invc[:, 32:64]
             neginvx = sm.tile([128, 32], F32, tag="neginvx")
             nc.vector.tensor_scalar(
                 out=neginvx[:, :], in0=invc[:, 0:32], scalar1=-1.0,
                 scalar2=None, op0=ALU.mult)
             twoinvx = sm.tile([128, 32], F32, tag="twoinvx")
             nc.vector.tensor_scalar(
                 out=twoinvx[:, :], in0=invc[:, 0:32], scalar1=2.0,
                 scalar2=None, op0=ALU.mult)

             # ---------------- broadcast invy along partitions -------------
             # col->DRAM (transposing dst AP), then DRAM->SBUF partition-bcast
             import concourse.bass as bass_mod
             nc.sync.dma_start(
                 out=scr_y[:, :].rearrange("r p -> p r"), in_=invc[:, 32:64])
             invybc = bc3.tile([128, HW], F32, tag="bcast")
             bcast_src_y = bass_mod.AP(
                 tensor=scr_y[:, :].tensor, offset=0, ap=[[0, 128], [1, HW]])
             nc.sync.dma_start(out=invybc[:, :], in_=bcast_src_y)

             # ---------------- y_n = y_c * invy (f32r) ---------------------
             for g in range(2):
                 nc.vector.tensor_tensor(
                     out=yn[:, g, :], in0=y[:, g, :], in1=invybc[:, :],
                     op=ALU.mult)

             # ---------------- PASS 1: row max + Z -------------------------
             gacc4 = sm.tile([128, 128], F32, tag="gacc4")
             zacc = sm.tile([128, 128], F32, tag="zacc")
             gmaxc = sm.tile([128, 32], F32, tag="gmaxc")
             reccol = sm.tile([128, 32], F32, tag="reccol")
             ab64 = sm.tile([128, 64], F32, tag="ab64")  # alpha | b2
             bcol = sm.tile([128, 32], F32, tag="bcol")
             tmpc = sm.tile([128, 32], F32, tag="tmpc")

             for r in range(NB):
                 pqs = [mmq.tile([128, 1024], F32, tag="pq", name=f"pq_{_i}")
                        for _i in range(NT)]
                 # stationary-major order: one weight per g, 8 chunks each
                 for g in range(2):
                     for c in range(8):
                         nc.tensor.matmul(
                             pqs[c // 2][:, 512 * (c % 2) : 512 * (c % 2 + 1)],
                             xc[:, g, 128 * r : 128 * (r + 1)],
                             yn[:, g, 512 * c : 512 * (c + 1)],
                             start=(g == 0), stop=(g == 1))
                 for q in range(NT):
                     nc.vector.reduce_max(
                         gacc4[:, 4 * r + q : 4 * r + q + 1], pqs[q][:, :],
                         axis=AX.X)
                 # alpha chain for this block
                 nc.vector.reduce_max(
                     gmaxc[:, r : r + 1],
                     gacc4[:, 4 * r : 4 * r + 4], axis=AX.X)
                 # dminE = (1+eps) - gmax*invx ; rec = 1/dminE
                 nc.vector.tensor_scalar(
                     out=tmpc[:, r : r + 1], in0=gmaxc[:, r : r + 1],
                     scalar1=neginvx[:, r : r + 1], scalar2=float(1.0 + EPS),
                     op0=ALU.mult, op1=ALU.add)
                 nc.vector.reciprocal(reccol[:, r : r + 1], tmpc[:, r : r + 1])
                 # alpha = 2*rec*invx ; b1 = 2 - 2*rec
                 nc.vector.tensor_scalar(
                     out=ab64[:, r : r + 1], in0=reccol[:, r : r + 1],
                     scalar1=twoinvx[:, r : r + 1], scalar2=None, op0=ALU.mult)
                 nc.vector.tensor_scalar(
                     out=bcol[:, r : r + 1], in0=reccol[:, r : r + 1],
                     scalar1=-2.0, scalar2=2.0, op0=ALU.mult, op1=ALU.add)
                 for q in range(NT):
                     nc.scalar.activation(
                         out=pqs[q][:, :], in_=pqs[q][:, :], func=ACTF.Exp,
                         bias=bcol[:, r : r + 1],
                         scale=ab64[:, r : r + 1],
                         accum_out=zacc[:, 4 * r + q : 4 * r + q + 1])

             # ---------------- interlude: b2 = b1 - lnZ; broadcasts --------
             zsum = sm.tile([128, 32], F32, tag="zsum")
             nc.vector.reduce_sum(
                 zsum[:, :], zacc[:, :].rearrange("p (r q) -> p r q", q=NT),
                 axis=AX.X)
             lnz = sm.tile([128, 32], F32, tag="lnz")
             nc.scalar.activation(out=lnz[:, :], in_=zsum[:, :], func=ACTF.Ln)
             nc.vector.tensor_tensor(
                 out=ab64[:, 32:64], in0=bcol[:, :], in1=lnz[:, :], op=ALU.subtract)

             # alpha/b2 cols -> DRAM rows (transposing dst), then bcast/row
             nc.sync.dma_start(
                 out=scr_a[:, :].rearrange("r p -> p r"), in_=ab64[:, 0:32])
             nc.sync.dma_start(
                 out=scr_b[:, :].rearrange("r p -> p r"), in_=ab64[:, 32:64])
             abc = bc3.tile([128, HW], F32, tag="bcast")
             bcast_src_a = bass_mod.AP(
                 tensor=scr_a[:, :].tensor, offset=0, ap=[[0, 128], [1, HW]])
             nc.sync.dma_start(out=abc[:, :], in_=bcast_src_a)
             b2row_f = bc3.tile([1, HW], F32, tag="bcast")
             nc.sync.dma_start(
                 out=b2row_f[0:1, :],
                 in_=scr_b[:, :].rearrange("r p -> (r p)"))
             b2row = bc3.tile([1, HW], F32R, tag="bcast")
             nc.vector.tensor_scalar(
                 out=b2row[:, :], in0=b2row_f[:, :], scalar1=1.0, scalar2=None,
                 op0=ALU.mult)

             # x2 = x_c * alpha  (in place, f32r)
             for g in range(2):
                 nc.vector.tensor_tensor(
                     out=xc[:, g, :], in0=xc[:, g, :].bitcast(F32),
                     in1=abc[:, :], op=ALU.mult)

             # ---------------- PASS 2: col max of A ------------------------
             macc = sm.tile([128, 128], F32, tag="macc")
             for rb in range(NB):
                 pqs = [mmq.tile([128, 1024], F32, tag="pq", name=f"pq_{_i}")
                        for _i in range(NT)]
                 # bias rows first (shared ones_row stationary), then one
                 # stationary per g for the 8 accumulating chunks.
                 for c in range(8):
                     nc.tensor.matmul(
                         pqs[c // 2][:, 512 * (c % 2) : 512 * (c % 2 + 1)],
                         ones_row_r[:, :],
                         b2row[:, 512 * c : 512 * (c + 1)],
                         start=True, stop=False)
                 for g in range(2):
                     for c in range(8):
                         nc.tensor.matmul(
                             pqs[c // 2][:, 512 * (c % 2) : 512 * (c % 2 + 1)],
                             yn[:, g, 128 * rb : 128 * (rb + 1)],
                             xc[:, g, 512 * c : 512 * (c + 1)],
                             start=False, stop=(g == 1))
                 for q in range(NT):
                     nc.vector.reduce_max(
                         macc[:, 4 * rb + q : 4 * rb + q + 1], pqs[q][:, :],
                         axis=AX.X)

             # ---------------- final ---------------------------------------
             mcol = sm.tile([128, 32], F32, tag="mcol")
             nc.vector.reduce_max(
                 mcol[:, :], macc[:, :].rearrange("p (r q) -> p r q", q=NT),
                 axis=AX.X)
             expm = sm.tile([128, 32], F32, tag="expm")
             csum = sm.tile([128, 1], F32, tag="csum")
             nc.scalar.activation(
                 out=expm[:, :], in_=mcol[:, :], func=ACTF.Exp,
                 accum_out=csum[:, :])
             tot = sm.tile([128, 1], F32, tag="tot")
             nc.gpsimd.partition_all_reduce(
                 tot[:, :], csum[:, :], channels=128,
                 reduce_op=bass_isa.ReduceOp.add)
             res = sm.tile([1, 1], F32, tag="res")
             nc.vector.tensor_scalar(
                 out=res[:, :], in0=tot[0:1, :], scalar1=float(1.0 / HW),
                 scalar2=None, op0=ALU.mult)
             nc.sync.dma_start(out=out[:, :], in_=res[:, :])
    nc.compile()
    return nc


def _get_nc():
    if "nc" not in _cached:
        _cached["nc"] = _build()
    return _cached["nc"]


def run_device(x, y, trace=False):
    """x, y: (N, C, H, W) float32. Returns (ccx (N,), BassKernelResults)."""
    x = np.ascontiguousarray(np.asarray(x, dtype=np.float32))
    y = np.ascontiguousarray(np.asarray(y, dtype=np.float32))
    ymu = y.mean(axis=(0, 2, 3), dtype=np.float64).astype(np.float32)  # (C,)
    ymu_arr = np.ascontiguousarray(ymu.reshape(2, 128).T)  # (128, 2)
    in_maps = []
    for n in range(N):
        in_maps.append({
            "xs": np.ascontiguousarray(x[n].reshape(C, HW)),
            "ys": np.ascontiguousarray(y[n].reshape(C, HW)),
            "ymu": ymu_arr,
        })
    nc = _get_nc()
    res = run_bass_kernel_spmd(nc, in_maps, core_ids=list(range(N)), trace=trace)
    ccx = np.array([res.results[n]["out"][0, 0] for n in range(N)], dtype=np.float32)
    return ccx, res


def kernel(x, y):
    ccx, _ = run_device(x, y)
    loss = float(np.mean(-np.log(ccx.astype(np.float64) + EPS)))
    return np.float32(loss)


if __name__ == "__main__":
    rng = np.random.default_rng(0)
    x = rng.standard_normal((N, C, H, W), dtype=np.float32)
    y = rng.standard_normal((N, C, H, W), dtype=np.float32)
    print("loss:", kernel(x, y))
